# revision 3
# baseline (speedup 1.0000x reference)
"""Trainium2 Bass kernel for nn_AdaptiveFusion.

Math (per batch b):
  q  = x @ Wq.T + bq                         (L,H)
  kv = g @ Wkv.T + bkv ; k,v = split         (Lg,H) each
  p  = softmax(q @ k.T / sqrt(H))            (L,Lg)
  gc = p @ v                                 (L,H)
  g1 = sigmoid(x @ W1x.T + gc @ W1y.T + bg1) (L,H)   [k-independent]
  h1 = gc + g1*(x - gc)                      (L,H)
  A  = h1 @ W2x.T                            (L,H)
  C  = s @ W2y.T + bg2                       (K,H)
  out[l,k,o] = s[k,o] + sigmoid(A[l,o]+C[k,o]) * (h1[l,o]-s[k,o])

Sharding: data-parallel over B (8 batches -> 8 cores), weights replicated,
no collectives.

Output stage (the bulk of the work: L*K*H = 6.3M elems/core) is split
across three engines to run concurrently:
  Pool (gpsimd): arg[:,k,:] = A[:,oc,:] + C[:,oc,k]     (tensor_scalar_add)
  ACT (scalar) : sig = Sigmoid(arg) in two large batched instrs per oc
  DVE (vector) : out = (h1 - s)*sig + s in ONE pass via a custom DVE op
                 (GATE_COMBINE_ANT, registered at import time)
Output DMA is written in [OC, 128, K, L] layout (exactly the SBUF tile
layout -> fully linear descriptors); the host permutes back.
"""

import os
import sys

import numpy as np

if "/opt/trn_rl_repo" not in sys.path:
    sys.path.insert(0, "/opt/trn_rl_repo")

import ml_dtypes

BF16 = ml_dtypes.bfloat16

B, L, K, Lg, H = 8, 256, 32, 128, 768
HC = H // 128  # h-chunks
OC = H // 128  # o-chunks

_CACHE = {}

last_exec_time_ns = None
last_profile = None


def _register_gate_combine():
    """Register the fused combine op: out = (in0 - s0)*in1 + s0.

    Additive runtime registration through the documented custom-DVE
    extension point (dve_ops.OPS); sha is computed from the lowered uops.
    """
    import concourse.dve_ops as dops

    if "GATE_COMBINE_ANT" in dops._SUB_OPCODE_FOR_NAME:
        return next(o for o in dops.OPS if o.name == "GATE_COMBINE_ANT")

    from concourse.dve_spec import C0, Spec, Src0, Src1, _has_src1, lower
    from concourse.dve_uop import DveOpSpec

    spec = Spec(
        body=(Src0 - C0) * Src1 + C0,
        reference=lambda in0, in1, s0, s1, imm2: (
            (in0.astype(np.float32) - s0) * in1 + s0
        ),
    )
    row = max(dops._SUB_OPCODE_FOR_NAME.values()) + 1
    shas = {}
    for ver in ("v3", "v4"):
        uops = lower(spec, ver=ver)
        s = DveOpSpec(
            name="GATE_COMBINE_ANT", opcode=row, uops=uops, rd1_en=_has_src1(spec)
        )
        shas[ver] = s.sha(ver)
    op = dops.DveOp("GATE_COMBINE_ANT", spec, subdim=False, uops_sha=shas)
    dops.OPS.append(op)
    dops.CUSTOM_DVE_SPECS[op.name] = spec
    dops._SUB_OPCODE_FOR_NAME[op.name] = row
    return op


def _build():
    import concourse.bacc as bacc
    import concourse.bass as bass
    import concourse.mybir as mybir
    import concourse.tile as tile

    gate_op = _register_gate_combine()

    f32 = mybir.dt.float32
    bf16 = mybir.dt.bfloat16
    AF = mybir.ActivationFunctionType
    OP = mybir.AluOpType

    nc = bacc.Bacc(None, target_bir_lowering=False, debug=False)

    # ---- DRAM parameters (per-core shard) ----
    xT = nc.declare_dram_parameter("xT", [H, L], bf16, isOutput=False)
    gT = nc.declare_dram_parameter("gT", [H, Lg], bf16, isOutput=False)
    sT = nc.declare_dram_parameter("sT", [H, K], bf16, isOutput=False)
    sTf = nc.declare_dram_parameter("sTf", [H, K], f32, isOutput=False)
    # weights stacked host-side in use order
    wstA = nc.declare_dram_parameter("wstA", [1, H, H], bf16, isOutput=False)  # wq
    wstB = nc.declare_dram_parameter("wstB", [2, H, H], bf16, isOutput=False)  # wk wv
    wstC = nc.declare_dram_parameter("wstC", [2, H, H], bf16, isOutput=False)  # w1x w1y
    wstE = nc.declare_dram_parameter("wstE", [1, H, H], bf16, isOutput=False)  # w2y
    wstF = nc.declare_dram_parameter("wstF", [1, H, H], bf16, isOutput=False)  # w2x
    # biases stacked host-side: [128, 5*OC] = bqs|bk|bv|bg1|bg2 chunks
    bstack = nc.declare_dram_parameter("bstack", [128, 5 * OC], f32, isOutput=False)
    ident = nc.declare_dram_parameter("ident", [128, 128], bf16, isOutput=False)
    out_d = nc.declare_dram_parameter("out", [OC, 128, K, L], bf16, isOutput=True)

    inv_sqrt_h = 1.0 / float(np.sqrt(H))

    with tile.TileContext(nc) as tc:
        with (
            tc.tile_pool(name="wpool", bufs=1) as wpool,
            tc.tile_pool(name="apool", bufs=1) as apool,
            tc.tile_pool(name="ppool", bufs=3, space=bass.MemorySpace.PSUM) as ppool,
            tc.tile_pool(name="vpool", bufs=1, space=bass.MemorySpace.PSUM) as vpool,
            tc.tile_pool(name="atp", bufs=1, space=bass.MemorySpace.PSUM) as atp,
            tc.tile_pool(name="spool", bufs=6) as spool,
            tc.tile_pool(name="opool", bufs=2) as opool,
        ):
            # ---- load weights & activations to SBUF (4 DMA queues) ----
            # scalar queue: small activations first (C-GEMM inputs), then x/g
            bst_s = apool.tile([128, 5 * OC], f32)
            nc.scalar.dma_start(bst_s[:], bstack[:])
            sT_s = apool.tile([128, HC, K], bf16)
            nc.scalar.dma_start(sT_s[:], sT[:].rearrange("(c p) l -> p c l", p=128))
            sTf_s = apool.tile([128, HC, K], f32)
            nc.scalar.dma_start(sTf_s[:], sTf[:].rearrange("(c p) l -> p c l", p=128))
            gT_s = apool.tile([128, HC, Lg], bf16)
            nc.scalar.dma_start(gT_s[:], gT[:].rearrange("(c p) l -> p c l", p=128))
            xT_s = apool.tile([128, HC, L], bf16)
            nc.scalar.dma_start(xT_s[:], xT[:].rearrange("(c p) l -> p c l", p=128))
            id_s = apool.tile([128, 128], bf16)
            nc.scalar.dma_start(id_s[:], ident[:])

            bqs_s, bk_s, bv_s = bst_s[:, 0:OC], bst_s[:, OC:2 * OC], bst_s[:, 2 * OC:3 * OC]
            bg1_s, bg2_s = bst_s[:, 3 * OC:4 * OC], bst_s[:, 4 * OC:5 * OC]

            # sync queue: w2y (C-GEMM) then wk/wv, wq
            wE_s = wpool.tile([128, 1, HC, H], bf16, tag="wE")
            nc.sync.dma_start(wE_s[:], wstE[:].rearrange("w (c p) o -> p w c o", p=128))
            wB_s = wpool.tile([128, 2, HC, H], bf16, tag="wB")
            nc.sync.dma_start(wB_s[:], wstB[:].rearrange("w (c p) o -> p w c o", p=128))
            wA_s = wpool.tile([128, 1, HC, H], bf16, tag="wA")
            nc.sync.dma_start(wA_s[:], wstA[:].rearrange("w (c p) o -> p w c o", p=128))
            # gpsimd queue: w1x/w1y then w2x
            wC_s = wpool.tile([128, 2, HC, H], bf16, tag="wC")
            nc.gpsimd.dma_start(wC_s[:], wstC[:].rearrange("w (c p) o -> p w c o", p=128))
            wF_s = wpool.tile([128, 1, HC, H], bf16, tag="wF")
            nc.gpsimd.dma_start(wF_s[:], wstF[:].rearrange("w (c p) o -> p w c o", p=128))

            wq_s = wA_s[:, 0]
            wk_s, wv_s = wB_s[:, 0], wB_s[:, 1]
            w1x_s, w1y_s = wC_s[:, 0], wC_s[:, 1]
            w2y_s, w2x_s = wE_s[:, 0], wF_s[:, 0]

            # warm the ACT tables (Exp + Sigmoid) while DMA streams in
            scratch = spool.tile([128, 2], f32, tag="warm")
            nc.scalar.activation(scratch[:, 0:1], bst_s[:, 0:1], AF.Exp)
            nc.scalar.activation(scratch[:, 1:2], bst_s[:, 0:1], AF.Sigmoid)

            # ---- C^T[o,k]+bg2 (first: gates the output stage) ----
            cb_s = apool.tile([128, OC, K], f32)
            for oc in range(OC):
                psc = ppool.tile([128, K], f32, tag="ps")
                for hc in range(HC):
                    nc.tensor.matmul(
                        psc[:], w2y_s[:, hc, oc * 128:(oc + 1) * 128], sT_s[:, hc, :],
                        start=(hc == 0), stop=(hc == HC - 1))
                nc.scalar.activation(cb_s[:, oc, :], psc[:], AF.Identity,
                                     bias=bg2_s[:, oc:oc + 1])

            # ---- k^T[o,m], v[m,o] ----
            kT_s = apool.tile([128, OC, Lg], bf16)
            for oc in range(OC):
                ps = ppool.tile([128, Lg], f32, tag="ps")
                for hc in range(HC):
                    nc.tensor.matmul(
                        ps[:], wk_s[:, hc, oc * 128:(oc + 1) * 128], gT_s[:, hc, :],
                        start=(hc == 0), stop=(hc == HC - 1))
                nc.scalar.activation(kT_s[:, oc, :], ps[:], AF.Identity,
                                     bias=bk_s[:, oc:oc + 1])

            # v (no bias; bv folded into gctx copy); stationary gT reused
            v_s = apool.tile([128, H], bf16)
            psv = vpool.tile([128, H], f32, tag="psv")
            for third in range(3):
                sl = slice(third * 256, (third + 1) * 256)
                for hc in range(HC):
                    nc.tensor.matmul(psv[:, sl], gT_s[:, hc, :], wv_s[:, hc, sl],
                                     start=(hc == 0), stop=(hc == HC - 1))
            nc.scalar.activation(v_s[:], psv[:], AF.Copy)

            # ---- q^T[o,l] ----
            qT_s = apool.tile([128, OC, L], bf16)
            for oc in range(OC):
                ps = ppool.tile([128, L], f32, tag="ps")
                for hc in range(HC):
                    nc.tensor.matmul(
                        ps[:], wq_s[:, hc, oc * 128:(oc + 1) * 128], xT_s[:, hc, :],
                        start=(hc == 0), stop=(hc == HC - 1))
                nc.scalar.activation(qT_s[:, oc, :], ps[:], AF.Identity,
                                     bias=bqs_s[:, oc:oc + 1], scale=inv_sqrt_h)

            # ---- scores + softmax + transpose(probs) ----
            probsT_s = apool.tile([128, 2, 128], bf16)  # [m, lb, l]
            for lb in range(2):
                pss = ppool.tile([128, Lg], f32, tag="ps")
                for oc in range(OC):
                    nc.tensor.matmul(
                        pss[:], qT_s[:, oc, lb * 128:(lb + 1) * 128], kT_s[:, oc, :],
                        start=(oc == 0), stop=(oc == OC - 1))
                nmax = spool.tile([128, 1], f32, tag="nmax")
                nc.vector.tensor_reduce(nmax[:], pss[:], mybir.AxisListType.X,
                                        OP.max, negate=True)
                e_s = spool.tile([128, Lg], bf16, tag="es")
                ssum = spool.tile([128, 1], f32, tag="ssum")
                nc.scalar.activation(e_s[:], pss[:], AF.Exp,
                                     bias=nmax[:], accum_out=ssum[:])
                rcp = spool.tile([128, 1], f32, tag="rcp")
                nc.vector.reciprocal(rcp[:], ssum[:])
                pr_s = spool.tile([128, Lg], bf16, tag="prs")
                nc.vector.tensor_scalar_mul(pr_s[:], e_s[:], rcp[:])
                pst = ppool.tile([128, 128], bf16, tag="ps")
                nc.tensor.transpose(pst[:], pr_s[:], id_s[:])
                nc.scalar.activation(probsT_s[:, lb, :], pst[:], AF.Copy)

            # ---- gctx^T[o,l] (bv folded in via bias) ----
            gcT_s = apool.tile([128, OC, L], bf16)
            for oc in range(OC):
                psg = ppool.tile([128, L], f32, tag="ps")
                for lb in range(2):
                    nc.tensor.matmul(
                        psg[:, lb * 128:(lb + 1) * 128],
                        v_s[:, oc * 128:(oc + 1) * 128], probsT_s[:, lb, :],
                        start=True, stop=True)
                nc.scalar.activation(gcT_s[:, oc, :], psg[:], AF.Identity,
                                     bias=bv_s[:, oc:oc + 1])

            # ---- gate1 + h1^T ----
            h1_s = apool.tile([128, HC, L], bf16)
            for oc in range(OC):
                ps1 = ppool.tile([128, L], f32, tag="ps")
                for hc in range(HC):
                    nc.tensor.matmul(
                        ps1[:], w1x_s[:, hc, oc * 128:(oc + 1) * 128], xT_s[:, hc, :],
                        start=(hc == 0), stop=False)
                for hc in range(HC):
                    nc.tensor.matmul(
                        ps1[:], w1y_s[:, hc, oc * 128:(oc + 1) * 128], gcT_s[:, hc, :],
                        start=False, stop=(hc == HC - 1))
                g1_s = spool.tile([128, L], bf16, tag="g1")
                nc.scalar.activation(g1_s[:], ps1[:], AF.Sigmoid,
                                     bias=bg1_s[:, oc:oc + 1])
                d1 = spool.tile([128, L], bf16, tag="d1")
                nc.vector.tensor_sub(d1[:], xT_s[:, oc, :], gcT_s[:, oc, :])
                m1 = spool.tile([128, L], bf16, tag="m1")
                nc.vector.tensor_mul(m1[:], d1[:], g1_s[:])
                nc.vector.tensor_add(h1_s[:, oc, :], m1[:], gcT_s[:, oc, :])

            # ---- A = h1 @ W2x per oc, then the 3-engine output pipeline ----
            A_sb = apool.tile([128, OC, L], bf16)
            at_p0 = atp.tile([128, 2, L], f32, tag="at0")
            at_p1 = atp.tile([128, 2, L], f32, tag="at1")
            at_p2 = atp.tile([128, 2, L], f32, tag="at2")
            at_tiles = [at_p0, at_p1, at_p2]
            KH = K // 2  # sigmoid batch: half the k's per ACT instruction

            for oc in range(OC):
                atv = at_tiles[oc // 2][:, oc % 2, :]
                for hc in range(HC):
                    nc.tensor.matmul(
                        atv, w2x_s[:, hc, oc * 128:(oc + 1) * 128],
                        h1_s[:, hc, :], start=(hc == 0), stop=(hc == HC - 1))
                nc.scalar.activation(A_sb[:, oc, :], atv, AF.Copy)

                arg = opool.tile([128, K, L], bf16, tag="arg")
                for k in range(K):
                    nc.gpsimd.tensor_scalar_add(
                        arg[:, k, :], A_sb[:, oc, :], cb_s[:, oc, k:k + 1])
                sig = opool.tile([128, K, L], bf16, tag="sig")
                for kh in range(2):
                    nc.scalar.activation(
                        sig[:, kh * KH:(kh + 1) * KH, :],
                        arg[:, kh * KH:(kh + 1) * KH, :], AF.Sigmoid)
                ob = opool.tile([128, K, L], bf16, tag="ob")
                for k in range(K):
                    nc.vector._custom_dve(
                        gate_op, out=ob[:, k, :], in0=h1_s[:, oc, :],
                        in1=sig[:, k, :], s0=sTf_s[:, oc, k:k + 1])
                eng = nc.sync if oc % 2 == 0 else nc.scalar
                eng.dma_start(out_d[oc], ob[:])

    nc.compile()
    return nc


def _prep_in_maps(x, s, g, Wq, bq, Wkv, bkv, Wg1, bg1, Wg2, bg2):
    def bT(a):  # transpose + bf16
        return np.ascontiguousarray(a.T).astype(BF16)

    def rsh(v):  # (H,) -> [128, H//128] partition-major chunks
        return np.ascontiguousarray(v.reshape(OC, 128).T).astype(np.float32)

    Wk, Wv = Wkv[:H], Wkv[H:]
    W1x, W1y = Wg1[:, :H], Wg1[:, H:]
    W2x, W2y = Wg2[:, :H], Wg2[:, H:]
    shared = {
        "wstA": np.stack([bT(Wq)]),
        "wstB": np.stack([bT(Wk), bT(Wv)]),
        "wstC": np.stack([bT(W1x), bT(W1y)]),
        "wstE": np.stack([bT(W2y)]),
        "wstF": np.stack([bT(W2x)]),
        "bstack": np.concatenate(
            [rsh(bq / np.sqrt(H)), rsh(bkv[:H]), rsh(bkv[H:]), rsh(bg1), rsh(bg2)],
            axis=1),
        "ident": np.eye(128, dtype=np.float32).astype(BF16),
    }
    in_maps = []
    for b in range(B):
        m = dict(shared)
        m["xT"] = bT(x[b])
        m["gT"] = bT(g[b])
        m["sT"] = bT(s[b])
        m["sTf"] = np.ascontiguousarray(s[b].T).astype(np.float32)
        in_maps.append(m)
    return in_maps


def kernel(**inputs):
    global last_exec_time_ns, last_profile
    from concourse.bass_utils import run_bass_kernel_spmd

    if "nc" not in _CACHE:
        _CACHE["nc"] = _build()
    nc = _CACHE["nc"]

    inputs = {k: np.asarray(v, dtype=np.float32) if np.asarray(v).dtype != np.int32
              else np.asarray(v) for k, v in inputs.items()}
    in_maps = _prep_in_maps(**inputs)

    trace = bool(int(os.environ.get("BASS_KERNEL_TRACE", "0")))
    repeat = int(os.environ.get("BASS_KERNEL_REPEAT", "1"))
    times = []
    for _ in range(repeat):
        res = run_bass_kernel_spmd(nc, in_maps, core_ids=list(range(B)), trace=trace)
        if res.exec_time_ns is not None:
            times.append(res.exec_time_ns)
    if times:
        print(f"exec times: {times}")
        last_exec_time_ns = min(times)
    last_profile = res.profile_json

    out = np.empty((B, L, K, H), dtype=np.float32)
    for b in range(B):
        # per-core result is [OC, 128, K, L] -> [L, K, H]
        r = res.results[b]["out"].astype(np.float32)
        out[b] = np.transpose(r, (3, 2, 0, 1)).reshape(L, K, H)
    return out


# revision 12
# speedup vs baseline: 4.3255x; 4.3255x over previous
"""Trainium2 Bass kernel for nn_AdaptiveFusion.

Math (per batch b):
  q  = x @ Wq.T + bq                         (L,H)
  kv = g @ Wkv.T + bkv ; k,v = split         (Lg,H) each
  p  = softmax(q @ k.T / sqrt(H))            (L,Lg)
  gc = p @ v                                 (L,H)
  g1 = sigmoid(x @ W1x.T + gc @ W1y.T + bg1) (L,H)   [k-independent]
  h1 = gc + g1*(x - gc)                      (L,H)
  A  = h1 @ W2x.T                            (L,H)
  C  = s @ W2y.T + bg2                       (K,H)
  out[l,k,o] = s[k,o] + sigmoid(A[l,o]+C[k,o]) * (h1[l,o]-s[k,o])

Sharding: data-parallel over B (8 batches -> 8 cores), weights replicated,
no collectives.

Output stage (the bulk of the work: L*K*H = 6.3M elems/core):
  ACT (scalar) : sig[:,k,:] = Sigmoid(A[:,oc,:] + C[:,oc,k]) per k,
                 reading A from PSUM with C as the per-partition bias
  DVE (vector) : d   = h1 - s      (TT over [128,K,64] tiles vs s_rep)
                 m   = d * sig     (one big [128,K*L] TT at 2x)
                 out = m + s_rep   (TT over [128,K,64] tiles)
                 All tensor_tensor at the 2x bf16 perf mode; s is
                 pre-replicated host-side along a 64-wide dummy-l axis
                 (pure layout prep) so no operand has stride-0 inner dims.
Output DMA is written in [OC, 128, K, L] layout (exactly the SBUF tile
layout -> fully linear descriptors); the host permutes back.
"""

import os
import sys

import numpy as np

if "/opt/trn_rl_repo" not in sys.path:
    sys.path.insert(0, "/opt/trn_rl_repo")

import ml_dtypes

BF16 = ml_dtypes.bfloat16

B, L, K, Lg, H = 8, 256, 32, 128, 768
HC = H // 128  # h-chunks
OC = H // 128  # o-chunks
LR = 64        # dummy-l width of the host-replicated s

_CACHE = {}

last_exec_time_ns = None
last_profile = None


def _build():
    import concourse.bacc as bacc
    import concourse.bass as bass
    import concourse.mybir as mybir
    import concourse.tile as tile

    f32 = mybir.dt.float32
    bf16 = mybir.dt.bfloat16
    AF = mybir.ActivationFunctionType
    OP = mybir.AluOpType

    nc = bacc.Bacc(None, target_bir_lowering=False, debug=False)

    # ---- DRAM parameters (per-core shard) ----
    xT = nc.declare_dram_parameter("xT", [H, L], bf16, isOutput=False)
    gT = nc.declare_dram_parameter("gT", [H, Lg], bf16, isOutput=False)
    sT = nc.declare_dram_parameter("sT", [H, K], bf16, isOutput=False)
    srep = nc.declare_dram_parameter("srep", [128, OC, K, LR], bf16, isOutput=False)
    # weights stacked host-side in use order
    wstA = nc.declare_dram_parameter("wstA", [1, H, H], bf16, isOutput=False)  # wq
    wstB = nc.declare_dram_parameter("wstB", [2, H, H], bf16, isOutput=False)  # wk wv
    wstC = nc.declare_dram_parameter("wstC", [2, H, H], bf16, isOutput=False)  # w1x w1y
    wstE = nc.declare_dram_parameter("wstE", [1, H, H], bf16, isOutput=False)  # w2y
    wstF = nc.declare_dram_parameter("wstF", [1, H, H], bf16, isOutput=False)  # w2x
    # biases stacked host-side: [128, 5*OC] = bqs|bk|bv|bg1|bg2 chunks
    bstack = nc.declare_dram_parameter("bstack", [128, 5 * OC], f32, isOutput=False)
    ident = nc.declare_dram_parameter("ident", [128, 128], bf16, isOutput=False)
    out_d = nc.declare_dram_parameter("out", [OC, 128, K, L], bf16, isOutput=True)

    inv_sqrt_h = 1.0 / float(np.sqrt(H))

    with tile.TileContext(nc) as tc:
        with (
            tc.tile_pool(name="wpool", bufs=1) as wpool,
            tc.tile_pool(name="apool", bufs=1) as apool,
            tc.tile_pool(name="ppool", bufs=3, space=bass.MemorySpace.PSUM) as ppool,
            tc.tile_pool(name="vpool", bufs=1, space=bass.MemorySpace.PSUM) as vpool,
            tc.tile_pool(name="atp", bufs=1, space=bass.MemorySpace.PSUM) as atp,
            tc.tile_pool(name="spool", bufs=6) as spool,
            tc.tile_pool(name="opool", bufs=2) as opool,
            tc.tile_pool(name="dpool", bufs=1) as dpool,
        ):
            # ---- load weights & activations to SBUF (4 DMA queues) ----
            # scalar queue: small activations first (C-GEMM inputs), then x/g
            bst_s = apool.tile([128, 5 * OC], f32)
            nc.scalar.dma_start(bst_s[:], bstack[:])
            sT_s = apool.tile([128, HC, K], bf16)
            nc.scalar.dma_start(sT_s[:], sT[:].rearrange("(c p) l -> p c l", p=128))
            srep_s = apool.tile([128, OC, K, LR], bf16)
            nc.scalar.dma_start(srep_s[:], srep[:])
            gT_s = apool.tile([128, HC, Lg], bf16)
            nc.scalar.dma_start(gT_s[:], gT[:].rearrange("(c p) l -> p c l", p=128))
            xT_s = apool.tile([128, HC, L], bf16)
            nc.scalar.dma_start(xT_s[:], xT[:].rearrange("(c p) l -> p c l", p=128))
            id_s = apool.tile([128, 128], bf16)
            nc.scalar.dma_start(id_s[:], ident[:])

            bqs_s, bk_s, bv_s = bst_s[:, 0:OC], bst_s[:, OC:2 * OC], bst_s[:, 2 * OC:3 * OC]
            bg1_s, bg2_s = bst_s[:, 3 * OC:4 * OC], bst_s[:, 4 * OC:5 * OC]

            # sync queue: w2y (C-GEMM) then wk/wv, wq
            wE_s = wpool.tile([128, 1, HC, H], bf16, tag="wE")
            nc.sync.dma_start(wE_s[:], wstE[:].rearrange("w (c p) o -> p w c o", p=128))
            wB_s = wpool.tile([128, 2, HC, H], bf16, tag="wB")
            nc.sync.dma_start(wB_s[:], wstB[:].rearrange("w (c p) o -> p w c o", p=128))
            wA_s = wpool.tile([128, 1, HC, H], bf16, tag="wA")
            nc.sync.dma_start(wA_s[:], wstA[:].rearrange("w (c p) o -> p w c o", p=128))
            # gpsimd queue: w1x/w1y then w2x
            wC_s = wpool.tile([128, 2, HC, H], bf16, tag="wC")
            nc.gpsimd.dma_start(wC_s[:], wstC[:].rearrange("w (c p) o -> p w c o", p=128))
            wF_s = wpool.tile([128, 1, HC, H], bf16, tag="wF")
            nc.gpsimd.dma_start(wF_s[:], wstF[:].rearrange("w (c p) o -> p w c o", p=128))

            wq_s = wA_s[:, 0]
            wk_s, wv_s = wB_s[:, 0], wB_s[:, 1]
            w1x_s, w1y_s = wC_s[:, 0], wC_s[:, 1]
            w2y_s, w2x_s = wE_s[:, 0], wF_s[:, 0]

            # warm the ACT tables (Exp + Sigmoid) while DMA streams in
            scratch = spool.tile([128, 2], f32, tag="warm")
            nc.scalar.activation(scratch[:, 0:1], bst_s[:, 0:1], AF.Exp)
            nc.scalar.activation(scratch[:, 1:2], bst_s[:, 0:1], AF.Sigmoid)

            # ---- C^T[o,k]+bg2 (first: gates the output stage) ----
            cb_s = apool.tile([128, OC, K], f32)
            for oc in range(OC):
                psc = ppool.tile([128, K], f32, tag="ps")
                for hc in range(HC):
                    nc.tensor.matmul(
                        psc[:], w2y_s[:, hc, oc * 128:(oc + 1) * 128], sT_s[:, hc, :],
                        start=(hc == 0), stop=(hc == HC - 1))
                nc.scalar.activation(cb_s[:, oc, :], psc[:], AF.Identity,
                                     bias=bg2_s[:, oc:oc + 1])

            # ---- k^T[o,m], v[m,o] ----
            kT_s = apool.tile([128, OC, Lg], bf16)
            for oc in range(OC):
                ps = ppool.tile([128, Lg], f32, tag="ps")
                for hc in range(HC):
                    nc.tensor.matmul(
                        ps[:], wk_s[:, hc, oc * 128:(oc + 1) * 128], gT_s[:, hc, :],
                        start=(hc == 0), stop=(hc == HC - 1))
                nc.scalar.activation(kT_s[:, oc, :], ps[:], AF.Identity,
                                     bias=bk_s[:, oc:oc + 1])

            # v (no bias; bv folded into gctx copy); stationary gT reused
            v_s = apool.tile([128, H], bf16)
            psv = vpool.tile([128, H], f32, tag="psv")
            for third in range(3):
                sl = slice(third * 256, (third + 1) * 256)
                for hc in range(HC):
                    nc.tensor.matmul(psv[:, sl], gT_s[:, hc, :], wv_s[:, hc, sl],
                                     start=(hc == 0), stop=(hc == HC - 1))
            nc.scalar.activation(v_s[:], psv[:], AF.Copy)

            # ---- q^T[o,l] ----
            qT_s = apool.tile([128, OC, L], bf16)
            for oc in range(OC):
                ps = ppool.tile([128, L], f32, tag="ps")
                for hc in range(HC):
                    nc.tensor.matmul(
                        ps[:], wq_s[:, hc, oc * 128:(oc + 1) * 128], xT_s[:, hc, :],
                        start=(hc == 0), stop=(hc == HC - 1))
                nc.scalar.activation(qT_s[:, oc, :], ps[:], AF.Identity,
                                     bias=bqs_s[:, oc:oc + 1], scale=inv_sqrt_h)

            # ---- scores + softmax + transpose(probs) ----
            probsT_s = apool.tile([128, 2, 128], bf16)  # [m, lb, l]
            for lb in range(2):
                pss = ppool.tile([128, Lg], f32, tag="ps")
                for oc in range(OC):
                    nc.tensor.matmul(
                        pss[:], qT_s[:, oc, lb * 128:(lb + 1) * 128], kT_s[:, oc, :],
                        start=(oc == 0), stop=(oc == OC - 1))
                nmax = spool.tile([128, 1], f32, tag="nmax")
                nc.vector.tensor_reduce(nmax[:], pss[:], mybir.AxisListType.X,
                                        OP.max, negate=True)
                e_s = spool.tile([128, Lg], bf16, tag="es")
                ssum = spool.tile([128, 1], f32, tag="ssum")
                nc.scalar.activation(e_s[:], pss[:], AF.Exp,
                                     bias=nmax[:], accum_out=ssum[:])
                rcp = spool.tile([128, 1], f32, tag="rcp")
                nc.vector.reciprocal(rcp[:], ssum[:])
                pr_s = spool.tile([128, Lg], bf16, tag="prs")
                nc.vector.tensor_scalar_mul(pr_s[:], e_s[:], rcp[:])
                pst = ppool.tile([128, 128], bf16, tag="ps")
                nc.tensor.transpose(pst[:], pr_s[:], id_s[:])
                nc.scalar.activation(probsT_s[:, lb, :], pst[:], AF.Copy)

            # ---- gctx^T[o,l] (bv folded in via bias) ----
            gcT_s = apool.tile([128, OC, L], bf16)
            for oc in range(OC):
                psg = ppool.tile([128, L], f32, tag="ps")
                for lb in range(2):
                    nc.tensor.matmul(
                        psg[:, lb * 128:(lb + 1) * 128],
                        v_s[:, oc * 128:(oc + 1) * 128], probsT_s[:, lb, :],
                        start=True, stop=True)
                nc.scalar.activation(gcT_s[:, oc, :], psg[:], AF.Identity,
                                     bias=bv_s[:, oc:oc + 1])

            # ---- gate1 + h1^T ----
            h1_s = apool.tile([128, HC, L], bf16)
            for oc in range(OC):
                ps1 = ppool.tile([128, L], f32, tag="ps")
                for hc in range(HC):
                    nc.tensor.matmul(
                        ps1[:], w1x_s[:, hc, oc * 128:(oc + 1) * 128], xT_s[:, hc, :],
                        start=(hc == 0), stop=False)
                for hc in range(HC):
                    nc.tensor.matmul(
                        ps1[:], w1y_s[:, hc, oc * 128:(oc + 1) * 128], gcT_s[:, hc, :],
                        start=False, stop=(hc == HC - 1))
                g1_s = spool.tile([128, L], bf16, tag="g1")
                nc.scalar.activation(g1_s[:], ps1[:], AF.Sigmoid,
                                     bias=bg1_s[:, oc:oc + 1])
                d1 = spool.tile([128, L], bf16, tag="d1")
                nc.vector.tensor_sub(d1[:], xT_s[:, oc, :], gcT_s[:, oc, :])
                m1 = spool.tile([128, L], bf16, tag="m1")
                nc.vector.tensor_mul(m1[:], d1[:], g1_s[:])
                nc.vector.tensor_add(h1_s[:, oc, :], m1[:], gcT_s[:, oc, :])

            # ---- A = h1 @ W2x per oc, then the output pipeline ----
            at_p0 = atp.tile([128, 2, L], f32, tag="at0")
            at_p1 = atp.tile([128, 2, L], f32, tag="at1")
            at_p2 = atp.tile([128, 2, L], f32, tag="at2")
            at_tiles = [at_p0, at_p1, at_p2]

            for oc in range(OC):
                atv = at_tiles[oc // 2][:, oc % 2, :]
                for hc in range(HC):
                    nc.tensor.matmul(
                        atv, w2x_s[:, hc, oc * 128:(oc + 1) * 128],
                        h1_s[:, hc, :], start=(hc == 0), stop=(hc == HC - 1))

                # sig[:,k,:] = sigmoid(A + C[:,k])  (ACT, per-k bias from PSUM)
                sig = opool.tile([128, K, L], bf16, tag="sig")
                for k in range(K):
                    nc.scalar.activation(sig[:, k, :], atv, AF.Sigmoid,
                                         bias=cb_s[:, oc, k:k + 1])
                # d = h1 - s (big TT at 2x against host-replicated s)
                dbuf = dpool.tile([128, K, L], bf16, tag="dbuf")
                for lc in range(L // LR):
                    nc.vector.tensor_sub(
                        dbuf[:, :, lc * LR:(lc + 1) * LR],
                        h1_s[:, oc, lc * LR:(lc + 1) * LR]
                        .unsqueeze(1).broadcast_to([128, K, LR]),
                        srep_s[:, oc])
                # m = d * sig (one big TT at 2x)
                ob = opool.tile([128, K, L], bf16, tag="ob")
                nc.vector.tensor_mul(
                    ob[:].rearrange("p k l -> p (k l)"),
                    dbuf[:].rearrange("p k l -> p (k l)"),
                    sig[:].rearrange("p k l -> p (k l)"))
                # out = m + s (big TT at 2x, in place)
                for lc in range(L // LR):
                    nc.vector.tensor_add(
                        ob[:, :, lc * LR:(lc + 1) * LR],
                        ob[:, :, lc * LR:(lc + 1) * LR],
                        srep_s[:, oc])
                eng = nc.sync if oc % 2 == 0 else nc.scalar
                eng.dma_start(out_d[oc], ob[:])

    nc.compile()
    return nc


def _prep_in_maps(x, s, g, Wq, bq, Wkv, bkv, Wg1, bg1, Wg2, bg2):
    def bT(a):  # transpose + bf16
        return np.ascontiguousarray(a.T).astype(BF16)

    def rsh(v):  # (H,) -> [128, H//128] partition-major chunks
        return np.ascontiguousarray(v.reshape(OC, 128).T).astype(np.float32)

    Wk, Wv = Wkv[:H], Wkv[H:]
    W1x, W1y = Wg1[:, :H], Wg1[:, H:]
    W2x, W2y = Wg2[:, :H], Wg2[:, H:]
    shared = {
        "wstA": np.stack([bT(Wq)]),
        "wstB": np.stack([bT(Wk), bT(Wv)]),
        "wstC": np.stack([bT(W1x), bT(W1y)]),
        "wstE": np.stack([bT(W2y)]),
        "wstF": np.stack([bT(W2x)]),
        "bstack": np.concatenate(
            [rsh(bq / np.sqrt(H)), rsh(bkv[:H]), rsh(bkv[H:]), rsh(bg1), rsh(bg2)],
            axis=1),
        "ident": np.eye(128, dtype=np.float32).astype(BF16),
    }
    in_maps = []
    for b in range(B):
        m = dict(shared)
        m["xT"] = bT(x[b])
        m["gT"] = bT(g[b])
        m["sT"] = bT(s[b])
        # [128, OC, K, LR]: s[k, oc*128+p] replicated along a dummy-l axis
        sr = np.asarray(s[b]).T.reshape(OC, 128, K).transpose(1, 0, 2)
        m["srep"] = np.ascontiguousarray(
            np.broadcast_to(sr[..., None], (128, OC, K, LR))).astype(BF16)
        in_maps.append(m)
    return in_maps


def kernel(**inputs):
    global last_exec_time_ns, last_profile
    from concourse.bass_utils import run_bass_kernel_spmd

    if "nc" not in _CACHE:
        _CACHE["nc"] = _build()
    nc = _CACHE["nc"]

    inputs = {k: np.asarray(v, dtype=np.float32) if np.asarray(v).dtype != np.int32
              else np.asarray(v) for k, v in inputs.items()}
    in_maps = _prep_in_maps(**inputs)

    trace = bool(int(os.environ.get("BASS_KERNEL_TRACE", "0")))
    repeat = int(os.environ.get("BASS_KERNEL_REPEAT", "1"))
    times = []
    for _ in range(repeat):
        res = run_bass_kernel_spmd(nc, in_maps, core_ids=list(range(B)), trace=trace)
        if res.exec_time_ns is not None:
            times.append(res.exec_time_ns)
    if times:
        print(f"exec times: {times}")
        last_exec_time_ns = min(times)
    last_profile = res.profile_json

    out = np.empty((B, L, K, H), dtype=np.float32)
    for b in range(B):
        # per-core result is [OC, 128, K, L] -> [L, K, H]
        r = res.results[b]["out"].astype(np.float32)
        out[b] = np.transpose(r, (3, 2, 0, 1)).reshape(L, K, H)
    return out


# revision 14
# speedup vs baseline: 4.6786x; 1.0817x over previous
"""Trainium2 Bass kernel for nn_AdaptiveFusion.

Math (per batch b):
  q  = x @ Wq.T + bq                         (L,H)
  kv = g @ Wkv.T + bkv ; k,v = split         (Lg,H) each
  p  = softmax(q @ k.T / sqrt(H))            (L,Lg)
  gc = p @ v                                 (L,H)
  g1 = sigmoid(x @ W1x.T + gc @ W1y.T + bg1) (L,H)   [k-independent]
  h1 = gc + g1*(x - gc)                      (L,H)
  A  = h1 @ W2x.T                            (L,H)
  C  = s @ W2y.T + bg2                       (K,H)
  out[l,k,o] = s[k,o] + sigmoid(A[l,o]+C[k,o]) * (h1[l,o]-s[k,o])

Sharding: data-parallel over B (8 batches -> 8 cores), weights replicated,
no collectives.

Output stage (L*K*H = 6.3M elems/core), balanced across ACT and DVE:
  k <  KF : sig = Sigmoid(A + C_k) per-k on ACT (bias trick, PSUM input)
  k >= KF : arg = A + C_rep via one DVE TT (2x), then one batched ACT sigmoid
  combine : d = h1 - s_rep ; m = d*sig ; out = m + s_rep -- all big
            tensor_tensor at the 2x bf16 perf mode. s is pre-replicated
            host-side along a 64-wide dummy-l axis (layout prep only) so
            no DVE operand has a stride-0 inner dim.
Output DMA is written in [OC, 128, K, L] layout (exactly the SBUF tile
layout -> fully linear descriptors); the host permutes back.
"""

import os
import sys

import numpy as np

if "/opt/trn_rl_repo" not in sys.path:
    sys.path.insert(0, "/opt/trn_rl_repo")

import ml_dtypes

BF16 = ml_dtypes.bfloat16

B, L, K, Lg, H = 8, 256, 32, 128, 768
HC = H // 128  # h-chunks
OC = H // 128  # o-chunks
LR = 64        # dummy-l width of the host-replicated s
KF = 24        # k's handled by fused per-k ACT sigmoid; rest via DVE arg
KS = K - KF

_CACHE = {}

last_exec_time_ns = None
last_profile = None


def _build():
    import concourse.bacc as bacc
    import concourse.bass as bass
    import concourse.mybir as mybir
    import concourse.tile as tile

    f32 = mybir.dt.float32
    bf16 = mybir.dt.bfloat16
    AF = mybir.ActivationFunctionType
    OP = mybir.AluOpType

    nc = bacc.Bacc(None, target_bir_lowering=False, debug=False)

    # ---- DRAM parameters (per-core shard) ----
    xT = nc.declare_dram_parameter("xT", [H, L], bf16, isOutput=False)
    gT = nc.declare_dram_parameter("gT", [H, Lg], bf16, isOutput=False)
    sT = nc.declare_dram_parameter("sT", [H, K], bf16, isOutput=False)
    srep = nc.declare_dram_parameter("srep", [128, OC, K, LR], bf16, isOutput=False)
    wq_d = nc.declare_dram_parameter("wq", [H, H], bf16, isOutput=False)
    wk_d = nc.declare_dram_parameter("wk", [H, H], bf16, isOutput=False)
    wv_d = nc.declare_dram_parameter("wv", [H, H], bf16, isOutput=False)
    w1x_d = nc.declare_dram_parameter("w1x", [H, H], bf16, isOutput=False)
    w1y_d = nc.declare_dram_parameter("w1y", [H, H], bf16, isOutput=False)
    w2x_d = nc.declare_dram_parameter("w2x", [H, H], bf16, isOutput=False)
    w2y_d = nc.declare_dram_parameter("w2y", [H, H], bf16, isOutput=False)
    bstack = nc.declare_dram_parameter("bstack", [128, 5 * OC], f32, isOutput=False)
    ident = nc.declare_dram_parameter("ident", [128, 128], bf16, isOutput=False)
    out_d = nc.declare_dram_parameter("out", [OC, 128, K, L], bf16, isOutput=True)

    inv_sqrt_h = 1.0 / float(np.sqrt(H))

    def wload(eng, name, src):
        t = wpool_ref[0].tile([128, HC, H], bf16, tag=name)
        eng.dma_start(t[:], src[:].rearrange("(c p) o -> p c o", p=128))
        return t

    with tile.TileContext(nc) as tc:
        with (
            tc.tile_pool(name="wpool", bufs=1) as wpool,
            tc.tile_pool(name="apool", bufs=1) as apool,
            tc.tile_pool(name="ppool", bufs=3, space=bass.MemorySpace.PSUM) as ppool,
            tc.tile_pool(name="vpool", bufs=1, space=bass.MemorySpace.PSUM) as vpool,
            tc.tile_pool(name="atp", bufs=1, space=bass.MemorySpace.PSUM) as atp,
            tc.tile_pool(name="spool", bufs=3) as spool,
            tc.tile_pool(name="opool", bufs=2) as opool,
            tc.tile_pool(name="dpool", bufs=1) as dpool,
        ):
            wpool_ref = [wpool]
            # ---- DMA queues, ordered by first use ----
            # sync: wk (scores chain), wq, wv
            wk_s = wload(nc.sync, "wk", wk_d)
            wq_s = wload(nc.sync, "wq", wq_d)
            wv_s = wload(nc.sync, "wv", wv_d)
            # gpsimd: w1x, w1y, w2x, then the replicated s
            w1x_s = wload(nc.gpsimd, "w1x", w1x_d)
            w1y_s = wload(nc.gpsimd, "w1y", w1y_d)
            w2x_s = wload(nc.gpsimd, "w2x", w2x_d)
            srep_s = apool.tile([128, OC, K, LR], bf16)
            nc.gpsimd.dma_start(srep_s[:], srep[:])
            # scalar: small tensors + activations + w2y (only needed by t~50us)
            bst_s = apool.tile([128, 5 * OC], f32)
            nc.scalar.dma_start(bst_s[:], bstack[:])
            gT_s = apool.tile([128, HC, Lg], bf16)
            nc.scalar.dma_start(gT_s[:], gT[:].rearrange("(c p) l -> p c l", p=128))
            xT_s = apool.tile([128, HC, L], bf16)
            nc.scalar.dma_start(xT_s[:], xT[:].rearrange("(c p) l -> p c l", p=128))
            sT_s = apool.tile([128, HC, K], bf16)
            nc.scalar.dma_start(sT_s[:], sT[:].rearrange("(c p) l -> p c l", p=128))
            id_s = apool.tile([128, 128], bf16)
            nc.scalar.dma_start(id_s[:], ident[:])
            w2y_s = wload(nc.scalar, "w2y", w2y_d)

            bqs_s, bk_s, bv_s = bst_s[:, 0:OC], bst_s[:, OC:2 * OC], bst_s[:, 2 * OC:3 * OC]
            bg1_s, bg2_s = bst_s[:, 3 * OC:4 * OC], bst_s[:, 4 * OC:5 * OC]

            # warm the ACT tables (Exp + Sigmoid) while DMA streams in
            scratch = spool.tile([128, 2], f32, tag="warm")
            nc.scalar.activation(scratch[:, 0:1], bst_s[:, 0:1], AF.Exp)
            nc.scalar.activation(scratch[:, 1:2], bst_s[:, 0:1], AF.Sigmoid)

            # ---- k^T[o,m] ----
            kT_s = apool.tile([128, OC, Lg], bf16)
            for oc in range(OC):
                ps = ppool.tile([128, Lg], f32, tag="ps")
                for hc in range(HC):
                    nc.tensor.matmul(
                        ps[:], wk_s[:, hc, oc * 128:(oc + 1) * 128], gT_s[:, hc, :],
                        start=(hc == 0), stop=(hc == HC - 1))
                nc.scalar.activation(kT_s[:, oc, :], ps[:], AF.Identity,
                                     bias=bk_s[:, oc:oc + 1])

            # ---- q^T[o,l] ----
            qT_s = apool.tile([128, OC, L], bf16)
            for oc in range(OC):
                ps = ppool.tile([128, L], f32, tag="ps")
                for hc in range(HC):
                    nc.tensor.matmul(
                        ps[:], wq_s[:, hc, oc * 128:(oc + 1) * 128], xT_s[:, hc, :],
                        start=(hc == 0), stop=(hc == HC - 1))
                nc.scalar.activation(qT_s[:, oc, :], ps[:], AF.Identity,
                                     bias=bqs_s[:, oc:oc + 1], scale=inv_sqrt_h)

            # ---- v[m,o] (no bias; bv folded into gctx copy) ----
            v_s = apool.tile([128, H], bf16)
            psv = vpool.tile([128, H], f32, tag="psv")
            for third in range(3):
                sl = slice(third * 256, (third + 1) * 256)
                for hc in range(HC):
                    nc.tensor.matmul(psv[:, sl], gT_s[:, hc, :], wv_s[:, hc, sl],
                                     start=(hc == 0), stop=(hc == HC - 1))
            nc.scalar.activation(v_s[:], psv[:], AF.Copy)

            # ---- scores + softmax + transpose(probs) ----
            probsT_s = apool.tile([128, 2, 128], bf16)  # [m, lb, l]
            for lb in range(2):
                pss = ppool.tile([128, Lg], f32, tag="ps")
                for oc in range(OC):
                    nc.tensor.matmul(
                        pss[:], qT_s[:, oc, lb * 128:(lb + 1) * 128], kT_s[:, oc, :],
                        start=(oc == 0), stop=(oc == OC - 1))
                nmax = spool.tile([128, 1], f32, tag="nmax")
                nc.vector.tensor_reduce(nmax[:], pss[:], mybir.AxisListType.X,
                                        OP.max, negate=True)
                e_s = spool.tile([128, Lg], bf16, tag="es")
                ssum = spool.tile([128, 1], f32, tag="ssum")
                nc.scalar.activation(e_s[:], pss[:], AF.Exp,
                                     bias=nmax[:], accum_out=ssum[:])
                rcp = spool.tile([128, 1], f32, tag="rcp")
                nc.vector.reciprocal(rcp[:], ssum[:])
                pr_s = spool.tile([128, Lg], bf16, tag="prs")
                nc.vector.tensor_scalar_mul(pr_s[:], e_s[:], rcp[:])
                pst = ppool.tile([128, 128], bf16, tag="ps")
                nc.tensor.transpose(pst[:], pr_s[:], id_s[:])
                nc.scalar.activation(probsT_s[:, lb, :], pst[:], AF.Copy)

            # ---- C^T[o,k]+bg2 (cb ready well before the sigmoids need it) ----
            cb_s = apool.tile([128, OC, K], f32)
            for oc in range(OC):
                psc = ppool.tile([128, K], f32, tag="ps")
                for hc in range(HC):
                    nc.tensor.matmul(
                        psc[:], w2y_s[:, hc, oc * 128:(oc + 1) * 128], sT_s[:, hc, :],
                        start=(hc == 0), stop=(hc == HC - 1))
                nc.scalar.activation(cb_s[:, oc, :], psc[:], AF.Identity,
                                     bias=bg2_s[:, oc:oc + 1])

            # ---- gctx^T[o,l] (bv folded in via bias) ----
            gcT_s = apool.tile([128, OC, L], bf16)
            for oc in range(OC):
                psg = ppool.tile([128, L], f32, tag="ps")
                for lb in range(2):
                    nc.tensor.matmul(
                        psg[:, lb * 128:(lb + 1) * 128],
                        v_s[:, oc * 128:(oc + 1) * 128], probsT_s[:, lb, :],
                        start=True, stop=True)
                nc.scalar.activation(gcT_s[:, oc, :], psg[:], AF.Identity,
                                     bias=bv_s[:, oc:oc + 1])

            # ---- gate1 + h1^T ----
            h1_s = apool.tile([128, HC, L], bf16)
            for oc in range(OC):
                ps1 = ppool.tile([128, L], f32, tag="ps")
                for hc in range(HC):
                    nc.tensor.matmul(
                        ps1[:], w1x_s[:, hc, oc * 128:(oc + 1) * 128], xT_s[:, hc, :],
                        start=(hc == 0), stop=False)
                for hc in range(HC):
                    nc.tensor.matmul(
                        ps1[:], w1y_s[:, hc, oc * 128:(oc + 1) * 128], gcT_s[:, hc, :],
                        start=False, stop=(hc == HC - 1))
                g1_s = spool.tile([128, L], bf16, tag="g1")
                nc.scalar.activation(g1_s[:], ps1[:], AF.Sigmoid,
                                     bias=bg1_s[:, oc:oc + 1])
                d1 = spool.tile([128, L], bf16, tag="d1")
                nc.vector.tensor_sub(d1[:], xT_s[:, oc, :], gcT_s[:, oc, :])
                m1 = spool.tile([128, L], bf16, tag="m1")
                nc.vector.tensor_mul(m1[:], d1[:], g1_s[:])
                nc.vector.tensor_add(h1_s[:, oc, :], m1[:], gcT_s[:, oc, :])

            # ---- A = h1 @ W2x per oc, then the output pipeline ----
            A_sb = apool.tile([128, OC, L], bf16)
            at_p0 = atp.tile([128, 2, L], f32, tag="at0")
            at_p1 = atp.tile([128, 2, L], f32, tag="at1")
            at_p2 = atp.tile([128, 2, L], f32, tag="at2")
            at_tiles = [at_p0, at_p1, at_p2]
            NLC = L // LR

            for oc in range(OC):
                atv = at_tiles[oc // 2][:, oc % 2, :]
                for hc in range(HC):
                    nc.tensor.matmul(
                        atv, w2x_s[:, hc, oc * 128:(oc + 1) * 128],
                        h1_s[:, hc, :], start=(hc == 0), stop=(hc == HC - 1))
                if KS:
                    nc.scalar.activation(A_sb[:, oc, :], atv, AF.Copy)

                # d = h1 - s first: keeps DVE busy while ACT runs sigmoids
                dbuf = dpool.tile([128, K, L], bf16, tag="dbuf")
                for lc in range(NLC):
                    nc.vector.tensor_sub(
                        dbuf[:, :, lc * LR:(lc + 1) * LR],
                        h1_s[:, oc, lc * LR:(lc + 1) * LR]
                        .unsqueeze(1).broadcast_to([128, K, LR]),
                        srep_s[:, oc])

                # sigmoids: k < KF fused on ACT; k >= KF via DVE arg + one ACT
                sig = opool.tile([128, K, L], bf16, tag="sig")
                for k in range(KF):
                    nc.scalar.activation(sig[:, k, :], atv, AF.Sigmoid,
                                         bias=cb_s[:, oc, k:k + 1])
                if KS:
                    crep = spool.tile([128, KS, LR], bf16, tag="crep")
                    nc.vector.tensor_copy(
                        crep[:], cb_s[:, oc, KF:].unsqueeze(2)
                        .broadcast_to([128, KS, LR]))
                    arg = dpool.tile([128, KS, L], bf16, tag="arg")
                    for lc in range(NLC):
                        nc.vector.tensor_add(
                            arg[:, :, lc * LR:(lc + 1) * LR],
                            A_sb[:, oc, lc * LR:(lc + 1) * LR]
                            .unsqueeze(1).broadcast_to([128, KS, LR]),
                            crep[:])
                    nc.scalar.activation(
                        sig[:, KF:, :].rearrange("p k l -> p (k l)"),
                        arg[:].rearrange("p k l -> p (k l)"), AF.Sigmoid)

                # m = d * sig in k-halves; out = m + s in (kh, lc) tiles
                ob = opool.tile([128, K, L], bf16, tag="ob")
                KH = K // 2
                for kh in range(2):
                    ksl = slice(kh * KH, (kh + 1) * KH)
                    nc.vector.tensor_mul(
                        ob[:, ksl, :].rearrange("p k l -> p (k l)"),
                        dbuf[:, ksl, :].rearrange("p k l -> p (k l)"),
                        sig[:, ksl, :].rearrange("p k l -> p (k l)"))
                    for lc in range(NLC):
                        nc.vector.tensor_add(
                            ob[:, ksl, lc * LR:(lc + 1) * LR],
                            ob[:, ksl, lc * LR:(lc + 1) * LR],
                            srep_s[:, oc, ksl])
                eng = nc.sync if oc % 2 == 0 else nc.gpsimd
                eng.dma_start(out_d[oc], ob[:])

    nc.compile()
    return nc


def _prep_in_maps(x, s, g, Wq, bq, Wkv, bkv, Wg1, bg1, Wg2, bg2):
    def bT(a):  # transpose + bf16
        return np.ascontiguousarray(a.T).astype(BF16)

    def rsh(v):  # (H,) -> [128, H//128] partition-major chunks
        return np.ascontiguousarray(v.reshape(OC, 128).T).astype(np.float32)

    Wk, Wv = Wkv[:H], Wkv[H:]
    W1x, W1y = Wg1[:, :H], Wg1[:, H:]
    W2x, W2y = Wg2[:, :H], Wg2[:, H:]
    shared = {
        "wq": bT(Wq), "wk": bT(Wk), "wv": bT(Wv),
        "w1x": bT(W1x), "w1y": bT(W1y), "w2x": bT(W2x), "w2y": bT(W2y),
        "bstack": np.concatenate(
            [rsh(bq / np.sqrt(H)), rsh(bkv[:H]), rsh(bkv[H:]), rsh(bg1), rsh(bg2)],
            axis=1),
        "ident": np.eye(128, dtype=np.float32).astype(BF16),
    }
    in_maps = []
    for b in range(B):
        m = dict(shared)
        m["xT"] = bT(x[b])
        m["gT"] = bT(g[b])
        m["sT"] = bT(s[b])
        # [128, OC, K, LR]: s[k, oc*128+p] replicated along a dummy-l axis
        sr = np.asarray(s[b]).T.reshape(OC, 128, K).transpose(1, 0, 2)
        m["srep"] = np.ascontiguousarray(
            np.broadcast_to(sr[..., None], (128, OC, K, LR))).astype(BF16)
        in_maps.append(m)
    return in_maps


def kernel(**inputs):
    global last_exec_time_ns, last_profile
    from concourse.bass_utils import run_bass_kernel_spmd

    if "nc" not in _CACHE:
        _CACHE["nc"] = _build()
    nc = _CACHE["nc"]

    inputs = {k: np.asarray(v, dtype=np.float32) if np.asarray(v).dtype != np.int32
              else np.asarray(v) for k, v in inputs.items()}
    in_maps = _prep_in_maps(**inputs)

    trace = bool(int(os.environ.get("BASS_KERNEL_TRACE", "0")))
    repeat = int(os.environ.get("BASS_KERNEL_REPEAT", "1"))
    times = []
    for _ in range(repeat):
        res = run_bass_kernel_spmd(nc, in_maps, core_ids=list(range(B)), trace=trace)
        if res.exec_time_ns is not None:
            times.append(res.exec_time_ns)
    if times:
        print(f"exec times: {times}")
        last_exec_time_ns = min(times)
    last_profile = res.profile_json

    out = np.empty((B, L, K, H), dtype=np.float32)
    for b in range(B):
        # per-core result is [OC, 128, K, L] -> [L, K, H]
        r = res.results[b]["out"].astype(np.float32)
        out[b] = np.transpose(r, (3, 2, 0, 1)).reshape(L, K, H)
    return out
